# revision 3
# baseline (speedup 1.0000x reference)
"""Trainium2 Bass kernel for nn_Attn: out = softmax(hidden @ (W @ objs + b)).

Key algebraic identity: energies = hidden @ (W @ objs + b) = (hidden @ W) @ objs + (hidden . b).
The (hidden . b) term is constant across objects, so softmax cancels it exactly.
Therefore we compute v = hidden @ W (a GEMV), then e = v @ objs (another GEMV),
then softmax(e) -- avoiding the [4096,4096] @ [4096,8192] GEMM entirely.

Sharding (8 cores): contraction dimension is sharded. Core i takes
  - W[:, 512*i : 512*(i+1)]      (computes v_i = hidden @ W_slice, 512 elements)
  - objs[512*i : 512*(i+1), :]   (computes partial energies e_i = v_i @ objs_slice)
Partial energies [8192] are AllReduce-summed across the 8 cores, then each core
computes the softmax redundantly; core 0's output is returned.

Per-core HBM traffic: 8MB (W slice) + 16MB (objs slice) ~= 24MB -> memory-bound
at ~360 GB/s per core.
"""

import functools
import os
import sys

sys.path.insert(0, "/opt/trn_rl_repo")

import numpy as np

H = 4096  # hidden size
N = 8192  # num objs
NCORES = 8
KS = H // NCORES  # 512 contraction rows per core

P = 128  # SBUF partitions
KT = H // P  # 32 k-tiles for the v = hidden @ W_slice matmuls
JT = KS // P  # 4 k-tiles for the e = v @ objs_slice matmuls
G = 4  # objs DMA groups (columns)
GN = N // G  # 2048 energy columns per group
S = GN // 512  # 4 matmul n-subtiles (512 wide) per group


@functools.lru_cache(maxsize=1)
def _build():
    import concourse.bass as bass
    import concourse.bacc as bacc
    import concourse.tile as tile
    import concourse.mybir as mybir

    f32 = mybir.dt.float32
    AX = mybir.AxisListType.X

    nc = bacc.Bacc(None, target_bir_lowering=False, debug=False, num_devices=NCORES)

    hidden_d = nc.dram_tensor("hidden", [H], f32, kind="ExternalInput")
    w_d = nc.dram_tensor("w_slice", [H, KS], f32, kind="ExternalInput")
    objs_d = nc.dram_tensor("objs_slice", [KS, N], f32, kind="ExternalInput")
    ident_d = nc.dram_tensor("ident", [P, P], f32, kind="ExternalInput")
    out_d = nc.dram_tensor("out", [1, N], f32, kind="ExternalOutput")

    with tile.TileContext(nc) as tc:
        with (
            tc.tile_pool(name="const", bufs=1) as constp,
            tc.tile_pool(name="wpool", bufs=1) as wpool,
            tc.tile_pool(name="opool", bufs=2) as opool,
            tc.tile_pool(name="sm", bufs=1) as smp,
            tc.tile_pool(name="dram", bufs=1, space=bass.MemorySpace.DRAM) as dramp,
            tc.tile_pool(name="ps_small", bufs=2, space=bass.MemorySpace.PSUM) as pssm,
            tc.tile_pool(name="ps_e", bufs=2, space=bass.MemorySpace.PSUM) as pse,
        ):
            # ---- constants / small inputs ----
            hid_sb = constp.tile([P, KT], f32)  # hid_sb[p, t] = hidden[t*128 + p]
            nc.sync.dma_start(hid_sb[:], hidden_d.ap().rearrange("(t p) -> p t", p=P))
            id_sb = constp.tile([P, P], f32)
            nc.sync.dma_start(id_sb[:], ident_d.ap())
            ones_row = constp.tile([1, P], f32)
            nc.vector.memset(ones_row[:], 1.0)
            ones_col = constp.tile([P, 1], f32)
            nc.vector.memset(ones_col[:], 1.0)

            # ---- W slice stream: w_sb[p, t, c] = W[t*128 + p, c] ----
            w_sb = wpool.tile([P, KT, KS], f32)  # 64KB/partition
            wap = w_d.ap().rearrange("(t p) c -> p t c", p=P)
            HKT = KT // 2
            nc.sync.dma_start(w_sb[:, :HKT, :], wap[:, :HKT, :])
            nc.sync.dma_start(w_sb[:, HKT:, :], wap[:, HKT:, :])

            # ---- v = hidden @ W_slice  -> [1, 512] in PSUM ----
            v_ps = pssm.tile([1, KS], f32, tag="ps")
            for t in range(KT):
                nc.tensor.matmul(
                    v_ps[:],
                    hid_sb[:, t : t + 1],
                    w_sb[:, t, :],
                    start=(t == 0),
                    stop=(t == KT - 1),
                )
            v_row = smp.tile([1, KS], f32)
            nc.vector.tensor_copy(v_row[:], v_ps[:])

            # ---- transpose v [1, 512] -> vT [128, 4] via K=1 matmuls ----
            # out[m, 0] = v_row[0, j*128 + m] * 1.0
            vT_sb = smp.tile([P, JT], f32)
            for j in range(JT):
                vT_ps = pssm.tile([P, 1], f32, tag="ps")
                nc.tensor.matmul(
                    vT_ps[:],
                    v_row[0:1, j * P : (j + 1) * P],
                    ones_row[0:1, 0:1],
                    start=True,
                    stop=True,
                )
                nc.vector.tensor_copy(vT_sb[:, j : j + 1], vT_ps[:])

            # ---- e_partial = v @ objs_slice -> [1, 8192], streamed in G groups ----
            # objs_ap[g, p, t, c] = objs_slice[t*128 + p, g*GN + c]
            objs_ap = objs_d.ap().rearrange("(t p) (g n) -> g p t n", p=P, n=GN)
            e_row = smp.tile([1, N], f32)
            for g in range(G):
                o_sb = opool.tile([P, JT, GN], f32)  # 32KB/partition
                nc.sync.dma_start(o_sb[:], objs_ap[g])
                for h in range(2):
                    e_ps = pse.tile([1, GN // 2], f32)  # 2 PSUM banks
                    for s2 in range(S // 2):
                        s = h * (S // 2) + s2
                        for t in range(JT):
                            nc.tensor.matmul(
                                e_ps[0:1, s2 * 512 : (s2 + 1) * 512],
                                vT_sb[:, t : t + 1],
                                o_sb[:, t, s * 512 : (s + 1) * 512],
                                start=(t == 0),
                                stop=(t == JT - 1),
                            )
                    nc.vector.tensor_copy(
                        e_row[0:1, g * GN + h * (GN // 2) : g * GN + (h + 1) * (GN // 2)],
                        e_ps[:],
                    )

            # ---- AllReduce partial energies across the 8 cores ----
            ar_in = dramp.tile([N], f32)
            ar_out = dramp.tile([N], f32)
            nc.sync.dma_start(ar_in[:].rearrange("(o n) -> o n", o=1), e_row[:])
            nc.gpsimd.collective_compute(
                "AllReduce",
                mybir.AluOpType.add,
                replica_groups=[list(range(NCORES))],
                ins=[ar_in.opt()],
                outs=[ar_out.opt()],
            )

            # ---- softmax over the full [8192] energies, laid out [128, 64] ----
            es = smp.tile([P, N // P], f32)
            nc.sync.dma_start(es[:], ar_out.rearrange("(p j) -> p j", p=P))

            rmax = smp.tile([P, 1], f32)
            nc.vector.reduce_max(rmax[:], es[:], axis=AX)
            # transpose rmax [128,1] -> [1,128] via identity matmul
            rt_ps = pssm.tile([1, P], f32, tag="ps")
            nc.tensor.matmul(rt_ps[:], rmax[:], id_sb[:], start=True, stop=True)
            rt_sb = smp.tile([1, P], f32)
            nc.vector.tensor_copy(rt_sb[:], rt_ps[:])
            gmax = smp.tile([1, 1], f32)
            nc.vector.reduce_max(gmax[:], rt_sb[:], axis=AX)
            ngmax = smp.tile([1, 1], f32)
            nc.vector.tensor_scalar_mul(ngmax[:], gmax[:], -1.0)
            # broadcast -gmax to all partitions: out[m,0] = ones_row[0,m] * ngmax
            nm_ps = pssm.tile([P, 1], f32, tag="ps")
            nc.tensor.matmul(nm_ps[:], ones_row[:], ngmax[:], start=True, stop=True)
            nmax_sb = smp.tile([P, 1], f32)
            nc.vector.tensor_copy(nmax_sb[:], nm_ps[:])

            exps = smp.tile([P, N // P], f32)
            nc.scalar.activation(
                exps[:],
                es[:],
                mybir.ActivationFunctionType.Exp,
                bias=nmax_sb[:],
            )

            rsum = smp.tile([P, 1], f32)
            nc.vector.reduce_sum(rsum[:], exps[:], axis=AX)
            tot_ps = pssm.tile([1, 1], f32, tag="ps")
            nc.tensor.matmul(tot_ps[:], rsum[:], ones_col[:], start=True, stop=True)
            tot_sb = smp.tile([1, 1], f32)
            nc.vector.tensor_copy(tot_sb[:], tot_ps[:])
            rcp = smp.tile([1, 1], f32)
            nc.vector.reciprocal(rcp[:], tot_sb[:])
            rc_ps = pssm.tile([P, 1], f32, tag="ps")
            nc.tensor.matmul(rc_ps[:], ones_row[:], rcp[:], start=True, stop=True)
            rcb_sb = smp.tile([P, 1], f32)
            nc.vector.tensor_copy(rcb_sb[:], rc_ps[:])

            out_sb = smp.tile([P, N // P], f32)
            nc.vector.tensor_scalar_mul(out_sb[:], exps[:], rcb_sb[:])
            nc.sync.dma_start(
                out_d.ap().rearrange("o (p j) -> (o p) j", p=P), out_sb[:]
            )

    nc.compile()
    return nc


def _in_maps(hidden, objs, W):
    hidden = np.ascontiguousarray(hidden, dtype=np.float32)
    ident = np.eye(P, dtype=np.float32)
    maps = []
    for i in range(NCORES):
        maps.append(
            {
                "hidden": hidden,
                "w_slice": np.ascontiguousarray(W[:, i * KS : (i + 1) * KS]),
                "objs_slice": np.ascontiguousarray(objs[i * KS : (i + 1) * KS, :]),
                "ident": ident,
            }
        )
    return maps


def kernel(hidden, objs, W, b, _trace=False):
    from concourse.bass_utils import run_bass_kernel_spmd

    nc = _build()
    res = run_bass_kernel_spmd(
        nc,
        _in_maps(hidden, objs, W),
        core_ids=list(range(NCORES)),
        trace=_trace,
    )
    out = res.results[0]["out"]
    if _trace:
        kernel.last_exec_time_ns = res.exec_time_ns
        kernel.last_results = res
    return np.asarray(out)


# revision 5
# speedup vs baseline: 1.0463x; 1.0463x over previous
"""Trainium2 Bass kernel for nn_Attn: out = softmax(hidden @ (W @ objs + b)).

Key algebraic identity: energies = hidden @ (W @ objs + b) = (hidden @ W) @ objs + (hidden . b).
The (hidden . b) term is constant across objects, so softmax cancels it exactly.
Therefore we compute v = hidden @ W (a GEMV), then e = v @ objs (another GEMV),
then softmax(e) -- avoiding the [4096,4096] @ [4096,8192] GEMM entirely.

Sharding (8 cores): contraction dimension is sharded. Core i takes
  - W[:, 512*i : 512*(i+1)]      (computes v_i = hidden @ W_slice, 512 elements)
  - objs[512*i : 512*(i+1), :]   (computes partial energies e_i = v_i @ objs_slice)
Partial energies [8192] are AllReduce-summed across the 8 cores, then each core
computes the softmax redundantly; core 0's output is returned.

Per-core HBM traffic: 8MB (W slice) + 16MB (objs slice) ~= 24MB -> memory-bound
at ~360 GB/s per core.
"""

import functools
import os
import sys

sys.path.insert(0, "/opt/trn_rl_repo")

import numpy as np

H = 4096  # hidden size
N = 8192  # num objs
NCORES = 8
KS = H // NCORES  # 512 contraction rows per core

P = 128  # SBUF partitions
KT = H // P  # 32 k-tiles for the v = hidden @ W_slice matmuls
JT = KS // P  # 4 k-tiles for the e = v @ objs_slice matmuls
G = 4  # objs DMA groups (columns)
GN = N // G  # 2048 energy columns per group
S = GN // 512  # 4 matmul n-subtiles (512 wide) per group


@functools.lru_cache(maxsize=1)
def _build():
    import concourse.bass as bass
    import concourse.bacc as bacc
    import concourse.tile as tile
    import concourse.mybir as mybir

    f32 = mybir.dt.float32
    f32r = mybir.dt.float32r
    AX = mybir.AxisListType.X

    nc = bacc.Bacc(None, target_bir_lowering=False, debug=False, num_devices=NCORES)

    hidden_d = nc.dram_tensor("hidden", [H], f32r, kind="ExternalInput")
    # Host pre-tiled layouts: w[p, t, c] = W_slice[t*128+p, c];
    # objs[p, g, t, c] = objs_slice[t*128+p, g*GN+c]
    w_d = nc.dram_tensor("w_slice", [P, KT, KS], f32r, kind="ExternalInput")
    objs_d = nc.dram_tensor("objs_slice", [P, G, JT, GN], f32r, kind="ExternalInput")
    ident_d = nc.dram_tensor("ident", [P, P], f32, kind="ExternalInput")
    out_d = nc.dram_tensor("out", [1, N], f32, kind="ExternalOutput")

    with tile.TileContext(nc) as tc:
        with (
            tc.tile_pool(name="const", bufs=1) as constp,
            tc.tile_pool(name="wpool", bufs=1) as wpool,
            tc.tile_pool(name="opool", bufs=2) as opool,
            tc.tile_pool(name="sm", bufs=1) as smp,
            tc.tile_pool(name="dram", bufs=1, space=bass.MemorySpace.DRAM) as dramp,
            tc.tile_pool(name="ps_small", bufs=2, space=bass.MemorySpace.PSUM) as pssm,
            tc.tile_pool(name="ps_e", bufs=2, space=bass.MemorySpace.PSUM) as pse,
        ):
            # ---- constants / small inputs ----
            hid_sb = constp.tile([P, KT], f32r)  # hid_sb[p, t] = hidden[t*128 + p]
            nc.sync.dma_start(hid_sb[:], hidden_d.ap().rearrange("(t p) -> p t", p=P))
            id_sb = constp.tile([P, P], f32)
            nc.sync.dma_start(id_sb[:], ident_d.ap())
            ones_row = constp.tile([1, P], f32)
            nc.vector.memset(ones_row[:], 1.0)
            ones_col = constp.tile([P, 1], f32)
            nc.vector.memset(ones_col[:], 1.0)

            # ---- W slice stream: w_sb[p, t, c] = W[t*128 + p, c] ----
            w_sb = wpool.tile([P, KT, KS], f32r)  # 64KB/partition
            wap = w_d.ap()
            HKT = KT // 2
            nc.sync.dma_start(w_sb[:, :HKT, :], wap[:, :HKT, :])
            nc.sync.dma_start(w_sb[:, HKT:, :], wap[:, HKT:, :])

            # ---- v = hidden @ W_slice  -> [1, 512] in PSUM ----
            v_ps = pssm.tile([1, KS], f32, tag="ps")
            for t in range(KT):
                nc.tensor.matmul(
                    v_ps[:],
                    hid_sb[:, t : t + 1],
                    w_sb[:, t, :],
                    start=(t == 0),
                    stop=(t == KT - 1),
                )
            v_row = smp.tile([1, KS], f32)
            nc.vector.tensor_copy(v_row[:], v_ps[:])

            # ---- transpose v [1, 512] -> vT [128, 4] via K=1 matmuls ----
            # out[m, 0] = v_row[0, j*128 + m] * 1.0
            vT_sb = smp.tile([P, JT], f32r)
            for j in range(JT):
                vT_ps = pssm.tile([P, 1], f32, tag="ps")
                nc.tensor.matmul(
                    vT_ps[:],
                    v_row[0:1, j * P : (j + 1) * P],
                    ones_row[0:1, 0:1],
                    start=True,
                    stop=True,
                )
                nc.vector.tensor_copy(vT_sb[:, j : j + 1], vT_ps[:])

            # ---- e_partial = v @ objs_slice -> [1, 8192], streamed in G groups ----
            # objs_ap[g, p, t, c] = objs_slice[t*128 + p, g*GN + c]
            objs_ap = objs_d.ap()
            e_row = smp.tile([1, N], f32)
            for g in range(G):
                o_sb = opool.tile([P, JT, GN], f32r)  # 32KB/partition
                nc.sync.dma_start(o_sb[:], objs_ap[:, g, :, :])
                for h in range(2):
                    e_ps = pse.tile([1, GN // 2], f32)  # 2 PSUM banks
                    for s2 in range(S // 2):
                        s = h * (S // 2) + s2
                        for t in range(JT):
                            nc.tensor.matmul(
                                e_ps[0:1, s2 * 512 : (s2 + 1) * 512],
                                vT_sb[:, t : t + 1],
                                o_sb[:, t, s * 512 : (s + 1) * 512],
                                start=(t == 0),
                                stop=(t == JT - 1),
                            )
                    nc.vector.tensor_copy(
                        e_row[0:1, g * GN + h * (GN // 2) : g * GN + (h + 1) * (GN // 2)],
                        e_ps[:],
                    )

            # ---- AllReduce partial energies across the 8 cores ----
            ar_in = dramp.tile([N], f32)
            ar_out = dramp.tile([N], f32)
            nc.sync.dma_start(ar_in[:].rearrange("(o n) -> o n", o=1), e_row[:])
            nc.gpsimd.collective_compute(
                "AllReduce",
                mybir.AluOpType.add,
                replica_groups=[list(range(NCORES))],
                ins=[ar_in.opt()],
                outs=[ar_out.opt()],
            )

            # ---- softmax over the full [8192] energies, laid out [128, 64] ----
            es = smp.tile([P, N // P], f32)
            nc.sync.dma_start(es[:], ar_out.rearrange("(p j) -> p j", p=P))

            rmax = smp.tile([P, 1], f32)
            nc.vector.reduce_max(rmax[:], es[:], axis=AX)
            # transpose rmax [128,1] -> [1,128] via identity matmul
            rt_ps = pssm.tile([1, P], f32, tag="ps")
            nc.tensor.matmul(rt_ps[:], rmax[:], id_sb[:], start=True, stop=True)
            rt_sb = smp.tile([1, P], f32)
            nc.vector.tensor_copy(rt_sb[:], rt_ps[:])
            gmax = smp.tile([1, 1], f32)
            nc.vector.reduce_max(gmax[:], rt_sb[:], axis=AX)
            ngmax = smp.tile([1, 1], f32)
            nc.vector.tensor_scalar_mul(ngmax[:], gmax[:], -1.0)
            # broadcast -gmax to all partitions: out[m,0] = ones_row[0,m] * ngmax
            nm_ps = pssm.tile([P, 1], f32, tag="ps")
            nc.tensor.matmul(nm_ps[:], ones_row[:], ngmax[:], start=True, stop=True)
            nmax_sb = smp.tile([P, 1], f32)
            nc.vector.tensor_copy(nmax_sb[:], nm_ps[:])

            exps = smp.tile([P, N // P], f32)
            nc.scalar.activation(
                exps[:],
                es[:],
                mybir.ActivationFunctionType.Exp,
                bias=nmax_sb[:],
            )

            rsum = smp.tile([P, 1], f32)
            nc.vector.reduce_sum(rsum[:], exps[:], axis=AX)
            tot_ps = pssm.tile([1, 1], f32, tag="ps")
            nc.tensor.matmul(tot_ps[:], rsum[:], ones_col[:], start=True, stop=True)
            tot_sb = smp.tile([1, 1], f32)
            nc.vector.tensor_copy(tot_sb[:], tot_ps[:])
            rcp = smp.tile([1, 1], f32)
            nc.vector.reciprocal(rcp[:], tot_sb[:])
            rc_ps = pssm.tile([P, 1], f32, tag="ps")
            nc.tensor.matmul(rc_ps[:], ones_row[:], rcp[:], start=True, stop=True)
            rcb_sb = smp.tile([P, 1], f32)
            nc.vector.tensor_copy(rcb_sb[:], rc_ps[:])

            out_sb = smp.tile([P, N // P], f32)
            nc.vector.tensor_scalar_mul(out_sb[:], exps[:], rcb_sb[:])
            nc.sync.dma_start(
                out_d.ap().rearrange("o (p j) -> (o p) j", p=P), out_sb[:]
            )

    nc.compile()
    return nc


def _in_maps(hidden, objs, W):
    hidden = np.ascontiguousarray(hidden, dtype=np.float32)
    ident = np.eye(P, dtype=np.float32)
    maps = []
    for i in range(NCORES):
        maps.append(
            {
                "hidden": hidden,
                "w_slice": np.ascontiguousarray(
                    W[:, i * KS : (i + 1) * KS].reshape(KT, P, KS).transpose(1, 0, 2)
                ),
                "objs_slice": np.ascontiguousarray(
                    objs[i * KS : (i + 1) * KS, :]
                    .reshape(JT, P, G, GN)
                    .transpose(1, 2, 0, 3)
                ),
                "ident": ident,
            }
        )
    return maps


def kernel(hidden, objs, W, b, _trace=False):
    from concourse.bass_utils import run_bass_kernel_spmd

    nc = _build()
    res = run_bass_kernel_spmd(
        nc,
        _in_maps(hidden, objs, W),
        core_ids=list(range(NCORES)),
        trace=_trace,
    )
    out = res.results[0]["out"]
    if _trace:
        kernel.last_exec_time_ns = res.exec_time_ns
        kernel.last_results = res
    return np.asarray(out)
